# revision 4
# baseline (speedup 1.0000x reference)
"""Character-LSTM Trainium2 kernel (8 NeuronCores, SPMD data-parallel).

Strategy
--------
All B*S = 16384 words run one batched LSTM recurrence. Work is split across 8
cores by dealing words (sorted by descending length) round-robin so every core
sees an identical per-step active-column count A[t]; within a core, words live
in SBUF as columns of transposed state tiles hT/cT [H x cols]. At step t only
the first A[t] columns are touched, so a word's last update lands exactly at
its final character and the surviving hT columns are the output. Short
word-length buckets are padded with dummy columns (char 0 -> zero embedding)
so A[t] is core-uniform and a multiple of 8.

Per step, gates are computed in transposed layout g[4H x cols] on the PE:
K-chunks = [embedded char x (128), h0 (128), h1 (128)] against bf16 weights,
accumulated fp32 in PSUM. The char embedding xT = emb.T @ onehot(chars) is a
2-matmul one-hot product (onehot built host-side from the index tensor);
sigma/tanh run on the scalar engine straight out of PSUM with the fused
per-partition bias; the cell update runs on the vector engine. c stays fp32,
h is stored bf16 (it is re-derived from fp32 c each step, so no error
accumulates through h storage).
"""

import sys

if "/opt/trn_rl_repo" not in sys.path:
    sys.path.insert(0, "/opt/trn_rl_repo")

import numpy as np
import ml_dtypes

import concourse.bass as bass
import concourse.tile as tile
from concourse import bacc, mybir
from concourse.bass import ts
from concourse.bass_utils import run_bass_kernel_spmd

BF16 = ml_dtypes.bfloat16
NCORES = 8
B, S, W, E, H, V = 64, 256, 24, 128, 256, 256
GATE_FUNCS = ["Sigmoid", "Sigmoid", "Tanh", "Sigmoid"]  # i, f, g, o
GROUP = 512  # matmul moving free-dim / PSUM bank width

_PROGRAM_CACHE: dict = {}


def _plan(lens: np.ndarray):
    """Column counts per step, uniform across cores, multiples of 8."""
    wL = np.bincount(lens, minlength=W + 1)
    colsL = np.zeros(W + 1, np.int64)
    cum = 0
    for L in range(W, 0, -1):
        need = -(-int(wL[L]) // NCORES)
        newcum = -(-(cum + need) // 8) * 8
        colsL[L] = newcum - cum
        cum = newcum
    C = max(cum, 8)
    # A[t] = number of active columns at step t = sum of cols for lengths > t
    A = [int(colsL[t + 1 :].sum()) for t in range(W)]
    return colsL, C, A


def _assign(lens, chars, colsL, C):
    """Deal words into (core, column) slots, longest first."""
    order = np.argsort(-lens, kind="stable")
    wL = np.bincount(lens, minlength=W + 1)
    colmap = np.full((NCORES, C), -1, np.int64)
    col_chars = np.zeros((NCORES, C, W), np.int64)
    pos = 0
    s = 0
    for L in range(W, 0, -1):
        cnt = int(wL[L])
        if cnt:
            ids = order[pos : pos + cnt]
            pos += cnt
            k = np.arange(cnt) % NCORES
            j = s + np.arange(cnt) // NCORES
            colmap[k, j] = ids
            col_chars[k, j] = chars[ids]
        s += int(colsL[L])
    return colmap, col_chars


def _build_program(C: int, A: list[int]):
    key = (C, tuple(A))
    if key in _PROGRAM_CACHE:
        return _PROGRAM_CACHE[key]

    dt = mybir.dt
    AF = mybir.ActivationFunctionType
    nc = bacc.Bacc("TRN2", target_bir_lowering=False, debug=False, num_devices=NCORES)

    oh_d = nc.dram_tensor("oh", [W, 2, 128, C], dt.bfloat16, kind="ExternalInput")
    emb_d = nc.dram_tensor("embc", [2, 128, E], dt.bfloat16, kind="ExternalInput")
    wih_d = nc.dram_tensor("wih", [E, 4 * H], dt.bfloat16, kind="ExternalInput")
    whh_d = nc.dram_tensor("whh", [2, 128, 4 * H], dt.bfloat16, kind="ExternalInput")
    bias_d = nc.dram_tensor("bias", [128, 8], dt.float32, kind="ExternalInput")
    out_d = nc.dram_tensor("out", [2, 128, C], dt.bfloat16, kind="ExternalOutput")

    xoff = np.concatenate([[0], np.cumsum(A)]).astype(int)  # offsets into xall
    XTOT = int(xoff[-1])

    with tile.TileContext(nc) as tc:
        with (
            tc.tile_pool(name="const", bufs=1) as constp,
            tc.tile_pool(name="state", bufs=1) as statep,
            tc.tile_pool(name="xall", bufs=1) as xallp,
            tc.tile_pool(name="oh", bufs=4) as ohp,
            tc.tile_pool(name="gates", bufs=3) as gatesp,
            tc.tile_pool(name="work", bufs=3) as workp,
            tc.tile_pool(name="psum", bufs=8, space="PSUM") as psump,
        ):
            emb_sb = [constp.tile([128, E], dt.bfloat16, tag=f"emb{v}", name=f"emb{v}") for v in range(2)]
            whh_sb = [constp.tile([128, 4 * H], dt.bfloat16, tag=f"whh{p}", name=f"whh{p}") for p in range(2)]
            wih_sb = constp.tile([E, 4 * H], dt.bfloat16, tag="wih")
            bias_sb = constp.tile([128, 8], dt.float32, tag="bias")
            for v in range(2):
                nc.sync.dma_start(out=emb_sb[v], in_=emb_d[v])
                nc.sync.dma_start(out=whh_sb[v], in_=whh_d[v])
            nc.sync.dma_start(out=wih_sb, in_=wih_d[:])
            nc.sync.dma_start(out=bias_sb, in_=bias_d[:])

            hbf = [statep.tile([128, C], dt.bfloat16, tag=f"h{p}", name=f"h{p}") for p in range(2)]
            cst = [statep.tile([128, C], dt.float32, tag=f"c{p}", name=f"c{p}") for p in range(2)]
            for p in range(2):
                nc.vector.memset(hbf[p][:], 0.0)
                nc.vector.memset(cst[p][:], 0.0)

            xall = xallp.tile([128, XTOT], dt.bfloat16, tag="xall")

            for t in range(W):
                At = A[t]
                if At == 0:
                    break
                ngroups = -(-At // GROUP)

                # --- x path: xT[t] = emb.T @ onehot(chars_t), all groups ---
                for g in range(ngroups):
                    a = min(GROUP, At - GROUP * g)
                    xps = psump.tile([128, a], dt.float32, tag="ps")
                    for v in range(2):
                        oht = ohp.tile([128, a], dt.bfloat16, tag=f"oh{v}")
                        nc.sync.dma_start(
                            out=oht,
                            in_=oh_d[t, v, :, GROUP * g : GROUP * g + a],
                        )
                        nc.tensor.matmul(
                            xps, emb_sb[v], oht, start=(v == 0), stop=(v == 1)
                        )
                    nc.scalar.copy(
                        out=xall[:, xoff[t] + GROUP * g : xoff[t] + GROUP * g + a],
                        in_=xps,
                    )

                # --- recurrence for step t ---
                for g in range(ngroups):
                    a = min(GROUP, At - GROUP * g)
                    lo, hi = GROUP * g, GROUP * g + a
                    xsl = xall[:, xoff[t] + lo : xoff[t] + hi]
                    gt = []
                    for m in range(8):
                        ps = psump.tile([128, a], dt.float32, tag="ps")
                        nc.tensor.matmul(
                            ps, wih_sb[:, ts(m, 128)], xsl, start=True, stop=False
                        )
                        nc.tensor.matmul(
                            ps,
                            whh_sb[0][:, ts(m, 128)],
                            hbf[0][:, lo:hi],
                            start=False,
                            stop=False,
                        )
                        nc.tensor.matmul(
                            ps,
                            whh_sb[1][:, ts(m, 128)],
                            hbf[1][:, lo:hi],
                            start=False,
                            stop=True,
                        )
                        gtile = gatesp.tile([128, a], dt.bfloat16, tag=f"g{m}", name=f"g{m}")
                        nc.scalar.activation(
                            gtile,
                            ps,
                            getattr(AF, GATE_FUNCS[m // 2]),
                            bias=bias_sb[:, m : m + 1],
                        )
                        gt.append(gtile)
                    for p in range(2):
                        ig = workp.tile([128, a], dt.bfloat16, tag=f"ig{p}")
                        nc.vector.tensor_mul(ig, gt[0 + p], gt[4 + p])
                        fc = workp.tile([128, a], dt.float32, tag=f"fc{p}")
                        nc.vector.tensor_mul(fc, gt[2 + p], cst[p][:, lo:hi])
                        nc.vector.tensor_add(cst[p][:, lo:hi], ig, fc)
                        th = workp.tile([128, a], dt.bfloat16, tag=f"th{p}")
                        nc.scalar.activation(th, cst[p][:, lo:hi], AF.Tanh)
                        nc.vector.tensor_mul(hbf[p][:, lo:hi], gt[6 + p], th)

            for p in range(2):
                nc.sync.dma_start(out=out_d[p], in_=hbf[p][:])

    nc.compile()
    _PROGRAM_CACHE[key] = nc
    return nc


def _prepare(char_input, embedding, W_ih, W_hh, b_ih, b_hh):
    ci = np.asarray(char_input)
    chars = ci.reshape(-1, W).astype(np.int64)
    lens = (chars != 0).sum(-1)

    colsL, C, A = _plan(lens)
    colmap, col_chars = _assign(lens, chars, colsL, C)

    emb_bf = np.ascontiguousarray(np.asarray(embedding).astype(BF16).reshape(2, 128, E))
    wih_bf = np.ascontiguousarray(np.asarray(W_ih).T.astype(BF16))  # [E, 4H]
    whh_bf = np.ascontiguousarray(
        np.asarray(W_hh).T.astype(BF16).reshape(2, 128, 4 * H)
    )  # [2,128,4H]
    bias_h = np.ascontiguousarray(
        (np.asarray(b_ih) + np.asarray(b_hh)).astype(np.float32).reshape(8, 128).T
    )  # [128, 8]

    vr = np.arange(V, dtype=np.int64)
    in_maps = []
    for k in range(NCORES):
        ids_t = col_chars[k].T  # [W, C]
        oh = (ids_t[:, :, None] == vr[None, None, :]).astype(BF16)  # [W, C, V]
        oh = np.ascontiguousarray(oh.transpose(0, 2, 1).reshape(W, 2, 128, C))
        in_maps.append(
            {
                "oh": oh,
                "embc": emb_bf,
                "wih": wih_bf,
                "whh": whh_bf,
                "bias": bias_h,
            }
        )
    return colmap, in_maps, C, A


def _gather_output(results, colmap):
    out_flat = np.zeros((B * S, H), np.float32)
    for k in range(NCORES):
        o = results[k]["out"].astype(np.float32)  # [2, 128, C]
        h_core = o.reshape(H, o.shape[-1])
        mask = colmap[k] >= 0
        out_flat[colmap[k][mask]] = h_core[:, mask].T
    return out_flat.reshape(B, S, H)


def kernel(char_input, embedding, W_ih, W_hh, b_ih, b_hh):
    colmap, in_maps, C, A = _prepare(char_input, embedding, W_ih, W_hh, b_ih, b_hh)
    nc = _build_program(C, A)
    res = run_bass_kernel_spmd(nc, in_maps, core_ids=list(range(NCORES)))
    return _gather_output(res.results, colmap)


# revision 6
# speedup vs baseline: 72.3010x; 72.3010x over previous
"""Character-LSTM Trainium2 kernel (8 NeuronCores, SPMD data-parallel).

Strategy
--------
All B*S = 16384 words run one batched LSTM recurrence. Work is split across 8
cores by dealing words (sorted by descending length) round-robin so every core
sees an identical per-step active-column count A[t]; within a core, words live
in SBUF as columns of transposed state tiles hT/cT [H x cols]. At step t only
the first A[t] columns are touched, so a word's last update lands exactly at
its final character and the surviving hT columns are the output. Short
word-length buckets are padded with dummy columns (char 0 -> zero embedding)
so A[t] is core-uniform and a multiple of 8.

Per step, gates are computed in transposed layout g[4H x cols] on the PE:
K-chunks = [embedded char x (128), h0 (128), h1 (128)] against bf16 weights,
accumulated fp32 in PSUM. The char embedding xT[:, j] = embT[:, char_j] is a
GPSIMD ap_gather from the on-chip embedding table (x_mode="gather"), or
alternatively a one-hot matmul (x_mode="onehot"); sigma/tanh run on the
scalar engine straight out of PSUM with the fused per-partition bias; the
cell update runs on the vector engine. c stays fp32, h is stored bf16 (it is
re-derived from fp32 c each step, so no error accumulates through h storage).
"""

import sys

if "/opt/trn_rl_repo" not in sys.path:
    sys.path.insert(0, "/opt/trn_rl_repo")

import numpy as np
import ml_dtypes

import concourse.bass as bass
import concourse.tile as tile
from concourse import bacc, mybir
from concourse.bass import ts
from concourse.bass_utils import run_bass_kernel_spmd

BF16 = ml_dtypes.bfloat16
NCORES = 8
B, S, W, E, H, V = 64, 256, 24, 128, 256, 256
GATE_FUNCS = ["Sigmoid", "Sigmoid", "Tanh", "Sigmoid"]  # i, f, g, o
GROUP = 512  # matmul moving free-dim / PSUM bank width
X_MODE = "gather"  # "gather" (gpsimd ap_gather) or "onehot" (PE matmul)

_PROGRAM_CACHE: dict = {}


def _plan(lens: np.ndarray):
    """Column counts per step, uniform across cores, multiples of 16."""
    wL = np.bincount(lens, minlength=W + 1)
    colsL = np.zeros(W + 1, np.int64)
    cum = 0
    for L in range(W, 0, -1):
        need = -(-int(wL[L]) // NCORES)
        newcum = -(-(cum + need) // 16) * 16
        colsL[L] = newcum - cum
        cum = newcum
    C = max(cum, 16)
    # A[t] = number of active columns at step t = sum of cols for lengths > t
    A = [int(colsL[t + 1 :].sum()) for t in range(W)]
    return colsL, C, A


def _assign(lens, chars, colsL, C):
    """Deal words into (core, column) slots, longest first."""
    order = np.argsort(-lens, kind="stable")
    wL = np.bincount(lens, minlength=W + 1)
    colmap = np.full((NCORES, C), -1, np.int64)
    col_chars = np.zeros((NCORES, C, W), np.int64)
    pos = 0
    s = 0
    for L in range(W, 0, -1):
        cnt = int(wL[L])
        if cnt:
            ids = order[pos : pos + cnt]
            pos += cnt
            k = np.arange(cnt) % NCORES
            j = s + np.arange(cnt) // NCORES
            colmap[k, j] = ids
            col_chars[k, j] = chars[ids]
        s += int(colsL[L])
    return colmap, col_chars


def _build_program(C: int, A: list[int], x_mode: str = X_MODE, reps: int = 1):
    key = (C, tuple(A), x_mode, reps)
    if key in _PROGRAM_CACHE:
        return _PROGRAM_CACHE[key]

    dt = mybir.dt
    AF = mybir.ActivationFunctionType
    nc = bacc.Bacc("TRN2", target_bir_lowering=False, debug=False, num_devices=NCORES)

    C16 = C // 16
    if x_mode == "gather":
        idx_d = nc.dram_tensor("idx", [W, 128, C16], dt.uint16, kind="ExternalInput")
        emb_d = nc.dram_tensor("embt", [128, V], dt.bfloat16, kind="ExternalInput")
    else:
        oh_d = nc.dram_tensor("oh", [W, 2, 128, C], dt.bfloat16, kind="ExternalInput")
        emb_d = nc.dram_tensor("embc", [2, 128, E], dt.bfloat16, kind="ExternalInput")
    wih_d = nc.dram_tensor("wih", [E, 4 * H], dt.bfloat16, kind="ExternalInput")
    whh_d = nc.dram_tensor("whh", [2, 128, 4 * H], dt.bfloat16, kind="ExternalInput")
    bias_d = nc.dram_tensor("bias", [128, 8], dt.float32, kind="ExternalInput")
    out_d = nc.dram_tensor("out", [2, 128, C], dt.bfloat16, kind="ExternalOutput")

    xoff = np.concatenate([[0], np.cumsum(A)]).astype(int)  # offsets into xall
    XTOT = int(xoff[-1])

    with tile.TileContext(nc) as tc:
        with (
            tc.tile_pool(name="const", bufs=1) as constp,
            tc.tile_pool(name="state", bufs=1) as statep,
            tc.tile_pool(name="xall", bufs=1) as xallp,
            tc.tile_pool(name="oh", bufs=4) as ohp,
            tc.tile_pool(name="gates", bufs=3) as gatesp,
            tc.tile_pool(name="work", bufs=3) as workp,
            tc.tile_pool(name="psum", bufs=8, space="PSUM") as psump,
        ):
            if x_mode == "gather":
                embt_sb = constp.tile([128, V], dt.bfloat16, tag="embt")
                nc.sync.dma_start(out=embt_sb, in_=emb_d[:])
            else:
                emb_sb = [
                    constp.tile([128, E], dt.bfloat16, tag=f"emb{v}", name=f"emb{v}")
                    for v in range(2)
                ]
                for v in range(2):
                    nc.sync.dma_start(out=emb_sb[v], in_=emb_d[v])
            whh_sb = [
                constp.tile([128, 4 * H], dt.bfloat16, tag=f"whh{p}", name=f"whh{p}")
                for p in range(2)
            ]
            wih_sb = constp.tile([E, 4 * H], dt.bfloat16, tag="wih")
            bias_sb = constp.tile([128, 8], dt.float32, tag="bias")
            for p in range(2):
                nc.sync.dma_start(out=whh_sb[p], in_=whh_d[p])
            nc.sync.dma_start(out=wih_sb, in_=wih_d[:])
            nc.sync.dma_start(out=bias_sb, in_=bias_d[:])

            hbf = [
                statep.tile([128, C], dt.bfloat16, tag=f"h{p}", name=f"h{p}")
                for p in range(2)
            ]
            cst = [
                statep.tile([128, C], dt.float32, tag=f"c{p}", name=f"c{p}")
                for p in range(2)
            ]
            xall = xallp.tile([128, XTOT], dt.bfloat16, tag="xall")

            import contextlib

            loop_cm = tc.For_i(0, reps, 1) if reps > 1 else contextlib.nullcontext()
            with loop_cm:
                for p in range(2):
                    nc.vector.memset(hbf[p][:], 0.0)
                    nc.vector.memset(cst[p][:], 0.0)

                for t in range(W):
                    At = A[t]
                    if At == 0:
                        break
                    ngroups = -(-At // GROUP)

                    # --- x path: xT[t][:, j] = embT[:, chars_t[j]] ---
                    for g in range(ngroups):
                        a = min(GROUP, At - GROUP * g)
                        xdst = xall[:, xoff[t] + GROUP * g : xoff[t] + GROUP * g + a]
                        if x_mode == "gather":
                            idxt = ohp.tile([128, GROUP // 16], dt.uint16, tag="idxt")
                            nc.sync.dma_start(
                                out=idxt[:, : a // 16],
                                in_=idx_d[t, :, GROUP * g // 16 : (GROUP * g + a) // 16],
                            )
                            nc.gpsimd.indirect_copy(
                                xdst,
                                embt_sb,
                                idxt[:, : a // 16],
                                i_know_ap_gather_is_preferred=True,
                            )
                        else:
                            xps = psump.tile([128, a], dt.float32, tag="ps")
                            for v in range(2):
                                oht = ohp.tile([128, a], dt.bfloat16, tag=f"oh{v}")
                                nc.sync.dma_start(
                                    out=oht,
                                    in_=oh_d[t, v, :, GROUP * g : GROUP * g + a],
                                )
                                nc.tensor.matmul(
                                    xps, emb_sb[v], oht, start=(v == 0), stop=(v == 1)
                                )
                            nc.scalar.copy(out=xdst, in_=xps)

                    # --- recurrence for step t ---
                    for g in range(ngroups):
                        a = min(GROUP, At - GROUP * g)
                        lo, hi = GROUP * g, GROUP * g + a
                        xsl = xall[:, xoff[t] + lo : xoff[t] + hi]
                        gt = []
                        for m in range(8):
                            ps = psump.tile([128, a], dt.float32, tag="ps")
                            nc.tensor.matmul(
                                ps, wih_sb[:, ts(m, 128)], xsl, start=True, stop=False
                            )
                            nc.tensor.matmul(
                                ps,
                                whh_sb[0][:, ts(m, 128)],
                                hbf[0][:, lo:hi],
                                start=False,
                                stop=False,
                            )
                            nc.tensor.matmul(
                                ps,
                                whh_sb[1][:, ts(m, 128)],
                                hbf[1][:, lo:hi],
                                start=False,
                                stop=True,
                            )
                            gtile = gatesp.tile(
                                [128, a], dt.bfloat16, tag=f"g{m}", name=f"g{m}"
                            )
                            nc.scalar.activation(
                                gtile,
                                ps,
                                getattr(AF, GATE_FUNCS[m // 2]),
                                bias=bias_sb[:, m : m + 1],
                            )
                            gt.append(gtile)
                        for p in range(2):
                            ig = workp.tile([128, a], dt.bfloat16, tag=f"ig{p}")
                            nc.vector.tensor_mul(ig, gt[0 + p], gt[4 + p])
                            fc = workp.tile([128, a], dt.float32, tag=f"fc{p}")
                            nc.vector.tensor_mul(fc, gt[2 + p], cst[p][:, lo:hi])
                            nc.vector.tensor_add(cst[p][:, lo:hi], ig, fc)
                            th = workp.tile([128, a], dt.bfloat16, tag=f"th{p}")
                            nc.scalar.activation(th, cst[p][:, lo:hi], AF.Tanh)
                            nc.vector.tensor_mul(hbf[p][:, lo:hi], gt[6 + p], th)

                for p in range(2):
                    nc.sync.dma_start(out=out_d[p], in_=hbf[p][:])

    nc.compile()
    _PROGRAM_CACHE[key] = nc
    return nc


def _prepare(char_input, embedding, W_ih, W_hh, b_ih, b_hh, x_mode: str = X_MODE):
    ci = np.asarray(char_input)
    chars = ci.reshape(-1, W).astype(np.int64)
    lens = (chars != 0).sum(-1)

    colsL, C, A = _plan(lens)
    colmap, col_chars = _assign(lens, chars, colsL, C)

    wih_bf = np.ascontiguousarray(np.asarray(W_ih).T.astype(BF16))  # [E, 4H]
    whh_bf = np.ascontiguousarray(
        np.asarray(W_hh).T.astype(BF16).reshape(2, 128, 4 * H)
    )  # [2,128,4H]
    bias_h = np.ascontiguousarray(
        (np.asarray(b_ih) + np.asarray(b_hh)).astype(np.float32).reshape(8, 128).T
    )  # [128, 8]

    common = {"wih": wih_bf, "whh": whh_bf, "bias": bias_h}
    in_maps = []
    if x_mode == "gather":
        embt = np.ascontiguousarray(np.asarray(embedding).T.astype(BF16))  # [128, V]
        for k in range(NCORES):
            # wrapped indices: output col j <- idxs[j % 16, j // 16], per 16-part group
            w = col_chars[k].T.reshape(W, C // 16, 16)  # [W, C/16, 16]
            w = np.ascontiguousarray(
                np.tile(w.transpose(0, 2, 1), (1, 8, 1))
            ).astype(np.uint16)  # [W, 128, C/16]
            in_maps.append({"idx": w, "embt": embt, **common})
    else:
        emb_bf = np.ascontiguousarray(
            np.asarray(embedding).astype(BF16).reshape(2, 128, E)
        )
        vr = np.arange(V, dtype=np.int64)
        for k in range(NCORES):
            ids_t = col_chars[k].T  # [W, C]
            oh = (ids_t[:, :, None] == vr[None, None, :]).astype(BF16)  # [W, C, V]
            oh = np.ascontiguousarray(oh.transpose(0, 2, 1).reshape(W, 2, 128, C))
            in_maps.append({"oh": oh, "embc": emb_bf, **common})
    return colmap, in_maps, C, A


def _gather_output(results, colmap):
    out_flat = np.zeros((B * S, H), np.float32)
    for k in range(NCORES):
        o = results[k]["out"].astype(np.float32)  # [2, 128, C]
        h_core = o.reshape(H, o.shape[-1])
        mask = colmap[k] >= 0
        out_flat[colmap[k][mask]] = h_core[:, mask].T
    return out_flat.reshape(B, S, H)


def kernel(char_input, embedding, W_ih, W_hh, b_ih, b_hh):
    colmap, in_maps, C, A = _prepare(char_input, embedding, W_ih, W_hh, b_ih, b_hh)
    nc = _build_program(C, A)
    res = run_bass_kernel_spmd(nc, in_maps, core_ids=list(range(NCORES)))
    return _gather_output(res.results, colmap)


# revision 8
# speedup vs baseline: 107.0783x; 1.4810x over previous
"""Character-LSTM Trainium2 kernel (8 NeuronCores, SPMD data-parallel).

Strategy
--------
All B*S = 16384 words run one batched LSTM recurrence. Work is split across 8
cores by dealing words (sorted by descending length) round-robin so every core
sees an identical per-step active-column count A[t]; within a core, words live
in SBUF as columns of transposed state tiles [H x cols]. At step t only the
first A[t] columns are touched, so a word's last update lands exactly at its
final character and the surviving h columns are the output. Short word-length
buckets are padded with dummy columns (char 0 -> zero embedding row) so A[t]
is core-uniform and a multiple of 16.

Per step, gates are computed in transposed layout g[4H x cols] on the PE as
one accumulation over four K=128 chunks: two one-hot chunks against the
per-vocab gate table emb_proj = W_ih @ emb[v] (precomputed on device), and two
h chunks against W_hh - all bf16 with fp32 PSUM accumulation. One-hots are
built on device by GPSIMD is_equal against a per-partition iota, from a
DMA-broadcast char row. Sigmoid/tanh run on the scalar engine straight out of
1024-wide PSUM reads with the fused per-partition bias. The cell update runs
on the vector engine in fp32; h keeps an fp32 master copy (the output) and a
GPSIMD-converted bf16 copy that feeds the next step's matmuls.
"""

import sys

if "/opt/trn_rl_repo" not in sys.path:
    sys.path.insert(0, "/opt/trn_rl_repo")

import contextlib

import numpy as np
import ml_dtypes

import concourse.bass as bass
import concourse.tile as tile
from concourse import bacc, mybir
from concourse.bass import ts
from concourse.bass_utils import run_bass_kernel_spmd

BF16 = ml_dtypes.bfloat16
NCORES = 8
B, S, W, E, H, V = 64, 256, 24, 128, 256, 256
GATE_FUNCS = ["Sigmoid", "Sigmoid", "Tanh", "Sigmoid"]  # i, f, g, o per 2 chunks
QW = 1024  # PSUM tile width (2 banks); ACT reads PSUM at line rate at this width
MM = 512  # matmul moving free-dim

_PROGRAM_CACHE: dict = {}


def _plan(lens: np.ndarray):
    """Column counts per step, uniform across cores, multiples of 16."""
    wL = np.bincount(lens, minlength=W + 1)
    colsL = np.zeros(W + 1, np.int64)
    cum = 0
    for L in range(W, 0, -1):
        need = -(-int(wL[L]) // NCORES)
        newcum = -(-(cum + need) // 16) * 16
        colsL[L] = newcum - cum
        cum = newcum
    C = max(cum, 16)
    A = [int(colsL[t + 1 :].sum()) for t in range(W)]
    return colsL, C, A


def _assign(lens, chars, colsL, C):
    """Deal words into (core, column) slots, longest first."""
    order = np.argsort(-lens, kind="stable")
    wL = np.bincount(lens, minlength=W + 1)
    colmap = np.full((NCORES, C), -1, np.int64)
    col_chars = np.zeros((NCORES, C, W), np.int64)
    pos = 0
    s = 0
    for L in range(W, 0, -1):
        cnt = int(wL[L])
        if cnt:
            ids = order[pos : pos + cnt]
            pos += cnt
            k = np.arange(cnt) % NCORES
            j = s + np.arange(cnt) // NCORES
            colmap[k, j] = ids
            col_chars[k, j] = chars[ids]
        s += int(colsL[L])
    return colmap, col_chars


def _build_program(C: int, A: list[int], reps: int = 1):
    key = (C, tuple(A), reps)
    if key in _PROGRAM_CACHE:
        return _PROGRAM_CACHE[key]

    dt = mybir.dt
    AF = mybir.ActivationFunctionType
    EQ = mybir.AluOpType.is_equal
    nc = bacc.Bacc("TRN2", target_bir_lowering=False, debug=False, num_devices=NCORES)

    chf_d = nc.dram_tensor("chf", [W, C], dt.bfloat16, kind="ExternalInput")
    embt_d = nc.dram_tensor("embt", [128, V], dt.bfloat16, kind="ExternalInput")
    wih_d = nc.dram_tensor("wih", [E, 4 * H], dt.bfloat16, kind="ExternalInput")
    whh_d = nc.dram_tensor("whh", [2, 128, 4 * H], dt.bfloat16, kind="ExternalInput")
    bias_d = nc.dram_tensor("bias", [128, 8], dt.float32, kind="ExternalInput")
    iota_d = nc.dram_tensor("iota", [128, 2], dt.float32, kind="ExternalInput")
    out_d = nc.dram_tensor("out", [2, 128, C], dt.float32, kind="ExternalOutput")

    with tile.TileContext(nc) as tc:
        with (
            tc.tile_pool(name="const", bufs=1) as constp,
            tc.tile_pool(name="state", bufs=1) as statep,
            tc.tile_pool(name="chp", bufs=3) as chp,
            tc.tile_pool(name="oh", bufs=3) as ohp,
            tc.tile_pool(name="gates", bufs=2) as gatesp,
            tc.tile_pool(name="work", bufs=3) as workp,
            tc.tile_pool(name="psum", bufs=4, space="PSUM") as psump,
        ):
            embt_sb = constp.tile([128, V], dt.bfloat16, tag="embt")
            wih_sb = constp.tile([E, 4 * H], dt.bfloat16, tag="wih")
            whh_sb = [
                constp.tile([128, 4 * H], dt.bfloat16, tag=f"whh{p}", name=f"whh{p}")
                for p in range(2)
            ]
            bias_sb = constp.tile([128, 8], dt.float32, tag="bias")
            iota_sb = constp.tile([128, 2], dt.float32, tag="iota")
            embproj_sb = [
                constp.tile([128, 4 * H], dt.bfloat16, tag=f"ep{v}", name=f"ep{v}")
                for v in range(2)
            ]
            nc.sync.dma_start(out=embt_sb, in_=embt_d[:])
            nc.sync.dma_start(out=wih_sb, in_=wih_d[:])
            for p in range(2):
                nc.sync.dma_start(out=whh_sb[p], in_=whh_d[p])
            nc.sync.dma_start(out=bias_sb, in_=bias_d[:])
            nc.sync.dma_start(out=iota_sb, in_=iota_d[:])

            hbf = [
                statep.tile([128, C], dt.bfloat16, tag=f"h{p}", name=f"h{p}")
                for p in range(2)
            ]
            hf = [
                statep.tile([128, C], dt.float32, tag=f"hf{p}", name=f"hf{p}")
                for p in range(2)
            ]
            cst = [
                statep.tile([128, C], dt.float32, tag=f"c{p}", name=f"c{p}")
                for p in range(2)
            ]

            loop_cm = tc.For_i(0, reps, 1) if reps > 1 else contextlib.nullcontext()
            with loop_cm:
                # emb_proj[v, :] = emb[v, :] @ W_ih.T  -> 2 chunk tiles [128, 4H]
                for v in range(2):
                    for hh in range(2):
                        pp = psump.tile([128, MM], dt.float32, tag="ps")
                        nc.tensor.matmul(
                            pp,
                            embt_sb[:, ts(v, 128)],
                            wih_sb[:, ts(hh, MM)],
                            start=True,
                            stop=True,
                        )
                        nc.scalar.copy(out=embproj_sb[v][:, ts(hh, MM)], in_=pp)

                for t in range(W):
                    At = A[t]
                    if At == 0:
                        break
                    At_next = A[t + 1] if t + 1 < W else 0
                    first = t == 0
                    ms = [0, 1, 4, 5, 6, 7] if first else list(range(8))
                    kchunks = 2 if first else 4

                    chrep = chp.tile([128, C], dt.bfloat16, tag="chrep")
                    src = chf_d[t, :At]
                    nc.sync.dma_start(
                        out=chrep[:, :At],
                        in_=bass.AP(
                            tensor=src.tensor, offset=src.offset,
                            ap=[[0, 128]] + list(src.ap),
                        ),
                    )

                    nq = -(-At // QW)
                    for q in range(nq):
                        qlo = QW * q
                        b = min(QW, At - qlo)
                        ohs = []
                        for v in range(2):
                            ohv = ohp.tile([128, QW], dt.bfloat16, tag=f"oh{v}", name=f"oh{v}")
                            nc.gpsimd.tensor_scalar(
                                ohv[:, :b],
                                chrep[:, qlo : qlo + b],
                                iota_sb[:, v : v + 1],
                                None,
                                op0=EQ,
                            )
                            ohs.append(ohv)
                        gt = []
                        for m in ms:
                            ps = psump.tile([128, QW], dt.float32, tag="ps")
                            for hh in range(-(-b // MM)):
                                w = min(MM, b - MM * hh)
                                osl = slice(MM * hh, MM * hh + w)
                                csl = slice(qlo + MM * hh, qlo + MM * hh + w)
                                nc.tensor.matmul(
                                    ps[:, osl],
                                    embproj_sb[0][:, ts(m, 128)],
                                    ohs[0][:, osl],
                                    start=True,
                                    stop=False,
                                )
                                nc.tensor.matmul(
                                    ps[:, osl],
                                    embproj_sb[1][:, ts(m, 128)],
                                    ohs[1][:, osl],
                                    start=False,
                                    stop=first,
                                )
                                if not first:
                                    nc.tensor.matmul(
                                        ps[:, osl],
                                        whh_sb[0][:, ts(m, 128)],
                                        hbf[0][:, csl],
                                        start=False,
                                        stop=False,
                                    )
                                    nc.tensor.matmul(
                                        ps[:, osl],
                                        whh_sb[1][:, ts(m, 128)],
                                        hbf[1][:, csl],
                                        start=False,
                                        stop=True,
                                    )
                            gtile = gatesp.tile(
                                [128, QW], dt.bfloat16, tag=f"g{m}", name=f"g{m}"
                            )
                            nc.scalar.activation(
                                gtile,
                                ps,
                                getattr(AF, GATE_FUNCS[m // 2]),
                                bias=bias_sb[:, m : m + 1],
                            )
                            gt.append(gtile)
                        gts = {m: g for m, g in zip(ms, gt)}
                        for p in range(2):
                            csl = cst[p][:, qlo : qlo + b]
                            if first:
                                nc.vector.tensor_mul(
                                    csl, gts[0 + p][:, :b], gts[4 + p][:, :b]
                                )
                            else:
                                ig = workp.tile([128, QW], dt.float32, tag=f"ig{p}", name=f"ig{p}")
                                nc.vector.tensor_mul(
                                    ig[:, :b], gts[0 + p][:, :b], gts[4 + p][:, :b]
                                )
                                fc = workp.tile([128, QW], dt.float32, tag=f"fc{p}", name=f"fc{p}")
                                nc.vector.tensor_mul(fc[:, :b], gts[2 + p][:, :b], csl)
                                nc.vector.tensor_add(csl, ig[:, :b], fc[:, :b])
                            th = workp.tile([128, QW], dt.bfloat16, tag=f"th{p}", name=f"th{p}")
                            nc.scalar.activation(th[:, :b], csl, AF.Tanh)
                            nc.vector.tensor_mul(
                                hf[p][:, qlo : qlo + b], gts[6 + p][:, :b], th[:, :b]
                            )
                            hb = min(qlo + b, At_next) - qlo
                            if hb > 0:
                                nc.gpsimd.tensor_copy(
                                    hbf[p][:, qlo : qlo + hb],
                                    hf[p][:, qlo : qlo + hb],
                                )

                for p in range(2):
                    nc.sync.dma_start(out=out_d[p], in_=hf[p][:])

    nc.compile()
    _PROGRAM_CACHE[key] = nc
    return nc


def _prepare(char_input, embedding, W_ih, W_hh, b_ih, b_hh):
    ci = np.asarray(char_input)
    chars = ci.reshape(-1, W).astype(np.int64)
    lens = (chars != 0).sum(-1)

    colsL, C, A = _plan(lens)
    colmap, col_chars = _assign(lens, chars, colsL, C)

    embt = np.ascontiguousarray(np.asarray(embedding).T.astype(BF16))  # [128, V]
    wih_bf = np.ascontiguousarray(np.asarray(W_ih).T.astype(BF16))  # [E, 4H]
    whh_bf = np.ascontiguousarray(
        np.asarray(W_hh).T.astype(BF16).reshape(2, 128, 4 * H)
    )
    bias_h = np.ascontiguousarray(
        (np.asarray(b_ih) + np.asarray(b_hh)).astype(np.float32).reshape(8, 128).T
    )
    iota = np.ascontiguousarray(
        (np.arange(128)[:, None] + np.array([0, 128])[None, :]).astype(np.float32)
    )

    common = {
        "embt": embt,
        "wih": wih_bf,
        "whh": whh_bf,
        "bias": bias_h,
        "iota": iota,
    }
    in_maps = []
    for k in range(NCORES):
        chf = np.ascontiguousarray(col_chars[k].T.astype(BF16))  # [W, C]
        in_maps.append({"chf": chf, **common})
    return colmap, in_maps, C, A


def _gather_output(results, colmap):
    out_flat = np.zeros((B * S, H), np.float32)
    for k in range(NCORES):
        o = results[k]["out"].astype(np.float32)  # [2, 128, C]
        h_core = o.reshape(H, o.shape[-1])
        mask = colmap[k] >= 0
        out_flat[colmap[k][mask]] = h_core[:, mask].T
    return out_flat.reshape(B, S, H)


def kernel(char_input, embedding, W_ih, W_hh, b_ih, b_hh):
    colmap, in_maps, C, A = _prepare(char_input, embedding, W_ih, W_hh, b_ih, b_hh)
    nc = _build_program(C, A)
    res = run_bass_kernel_spmd(nc, in_maps, core_ids=list(range(NCORES)))
    return _gather_output(res.results, colmap)
